# revision 20
# baseline (speedup 1.0000x reference)
"""GroupQuantLinear on 8 Trainium2 NeuronCores.

y[b,s,o] = x[b,s,:] @ W[o,:] + bias[o], where W is dequantized on-device from
4-bit packed weights with per-(o, group) affine scale/bias (groups of 256 along
the 4096-wide input dim).

Sharding: tensor-parallel on out_features (8 shards of 2048 rows); x replicated.

Per-core kernel (Bass/Tile), v3:
  The packed words only use bits 0..15, so the host passes them as uint16
  [OSH, NW].  Per 1024-col half of OSH:
    - XBAR DMA-transpose (DRAM->SBUF, 8 big [1024,128] u16 chunks) gives
      w16t [word, o] tiles directly in matmul orientation -- no PE transposes
      and no transposition of the 4x larger dequantized data.
    - DVE unpack (shift+and, u16) + 2 broadcast-AP tensor_tensor ops apply the
      per-(o, group) affine dequant: in the [word, o] orientation the group is
      a function of the partition band, so the host pre-replicates scale/bias
      into banded [128, 8wt, OSH] bf16 tensors (layout-only np.broadcast).
      Output goes straight into the SBUF-resident WT [128, 32kt, 1024] bf16.
    - matmul: per m-tile, DMA x^T slice (f32), cast bf16, 64 back-to-back
      [128k,128m]x[128k,512o] matmuls.  The k-accumulation runs in wt-major
      order (kt = plane*8 + wt) so the PE can start as soon as the first
      word-tile is dequantized.  Evict = one DVE add of broadcast bias.

Host marshalling is layout-only: x is transposed/permuted so the contraction
dim lands on SBUF partitions in the same nibble-plane-major order the on-chip
unpack produces (in' = plane*1024 + word); packed words are viewed as uint16;
scale/bias are transposed + replicated into the banded layout.
"""

import numpy as np

B, S, IN, OUT, G = 2, 2048, 4096, 16384, 16
NCORES = 8
OSH = OUT // NCORES       # 2048 out rows per core
BS = B * S                # 4096
NW = IN // 4              # 1024 packed words per out row
P = 128

_COMPILED = {}


def _build_nc():
    from contextlib import ExitStack

    import concourse.bass as bass
    import concourse.mybir as mybir
    import concourse.tile as tile
    from concourse import bacc
    from concourse.bass import ds, ts

    f32 = mybir.dt.float32
    bf16 = mybir.dt.bfloat16
    u16 = mybir.dt.uint16

    nc = bacc.Bacc(None, target_bir_lowering=False)

    xtp = nc.dram_tensor("xtp", [IN, BS], f32, kind="ExternalInput")
    # packed words pre-transposed on host: [word, o]
    wpk = nc.dram_tensor("wpk", [NW, OSH], u16, kind="ExternalInput")
    # [half, 128, 8, HALF] banded scale/bias (bands of 64 partitions per group)
    sbc_d = nc.dram_tensor("sbc", [2, P, 8, OSH // 2], bf16, kind="ExternalInput")
    bbc_d = nc.dram_tensor("bbc", [2, P, 8, OSH // 2], bf16, kind="ExternalInput")
    bias = nc.dram_tensor("bias", [1, OSH], f32, kind="ExternalInput")
    y = nc.dram_tensor("y", [BS, OSH], f32, kind="ExternalOutput")

    NKT = IN // P             # 32 k-subtiles
    NWT = 8                   # word-tiles of 128 words
    HALF = OSH // 2           # 1024 o-cols per half
    N_MT = BS // P            # 32 m-tiles
    N_NT = HALF // 512        # 2 n-tiles of 512 per half

    with tile.TileContext(nc) as tc:
        with ExitStack() as ctx:
            const = ctx.enter_context(tc.tile_pool(name="const", bufs=1))
            wt_pool = ctx.enter_context(tc.tile_pool(name="wt", bufs=1))
            sb_pool = ctx.enter_context(tc.tile_pool(name="sb", bufs=1))
            w16_pool = ctx.enter_context(tc.tile_pool(name="w16", bufs=9))
            q4_pool = ctx.enter_context(tc.tile_pool(name="q4", bufs=3))
            tmp_pool = ctx.enter_context(tc.tile_pool(name="tmp", bufs=3))
            xf_pool = ctx.enter_context(tc.tile_pool(name="xf", bufs=3))
            xb_pool = ctx.enter_context(tc.tile_pool(name="xb", bufs=2))
            ev_pool = ctx.enter_context(tc.tile_pool(name="ev", bufs=3))
            psum = ctx.enter_context(tc.tile_pool(name="psum", bufs=4, space="PSUM"))

            bias_sb = const.tile([1, OSH], f32)
            nc.sync.dma_start(bias_sb[:], bias[:])
            ones_sb = const.tile([1, P], f32)
            nc.any.memset(ones_sb[:], 1.0)
            bias_bc = const.tile([P, OSH], f32)
            for j in range(OSH // 512):
                bps = psum.tile([P, 512], f32, tag="ps")
                nc.tensor.matmul(
                    bps[:], ones_sb[:], bias_sb[:, ts(j, 512)], start=True, stop=True
                )
                nc.any.tensor_copy(bias_bc[:, ts(j, 512)], bps[:])

            # x^T viewed as [p, kt, m] with k-subtile on partitions
            xv = xtp.rearrange("(kt p) m -> p kt m", p=P)

            # wt-major k accumulation order: kt = plane*8 + wt
            korder = [plane * NWT + wt for wt in range(NWT) for plane in range(4)]

            for half in range(2):
                osl = ds(half * HALF, HALF)
                # W^T for this half: [word-in-tile, kt, o] bf16
                wt_t = wt_pool.tile([P, NKT, HALF], bf16, tag="wt")
                sbc = sb_pool.tile([P, NWT, HALF], bf16, tag="sbc")
                bbc = sb_pool.tile([P, NWT, HALF], bf16, tag="bbc")

                # ---- dequant: 8 word-tiles into wt_t, n=0 o-chunk first ----
                w16s = []
                for wt in range(NWT):
                    nc.sync.dma_start(sbc[:, wt, :], sbc_d[half, :, wt, :])
                    nc.sync.dma_start(bbc[:, wt, :], bbc_d[half, :, wt, :])
                    w16 = w16_pool.tile([P, HALF], u16, tag="w16")
                    w16s.append(w16)
                    nc.sync.dma_start(w16[:], wpk[ts(wt, P), osl])
                for oc in range(N_NT):
                    ocs = ts(oc, 512)
                    for wt in range(NWT):
                        q4 = q4_pool.tile([P, 4, 512], u16, tag="q4")
                        for k in range(4):
                            nc.vector.tensor_scalar(
                                q4[:, k, :],
                                w16s[wt][:, ocs],
                                4 * k,
                                0xF,
                                mybir.AluOpType.logical_shift_right,
                                mybir.AluOpType.bitwise_and,
                            )
                        tmp = tmp_pool.tile([P, 4, 512], bf16, tag="tmp")
                        nc.vector.tensor_tensor(
                            tmp[:],
                            q4[:],
                            sbc[:, wt, None, ocs].to_broadcast((P, 4, 512)),
                            mybir.AluOpType.mult,
                        )
                        # kt slices for this wt: plane*8 + wt
                        nc.gpsimd.tensor_tensor(
                            wt_t[:, wt :: NWT, ocs],
                            tmp[:],
                            bbc[:, wt, None, ocs].to_broadcast((P, 4, 512)),
                            mybir.AluOpType.add,
                        )

                # ---- matmul y[:, half] = x @ WT + bias ----
                for mt in range(N_MT):
                    msl = ts(mt, P)
                    xf_a = xf_pool.tile([P, NKT // 2, P], f32, tag="xf")
                    nc.sync.dma_start(xf_a[:], xv[:, : NKT // 2, msl])
                    xf_b = xf_pool.tile([P, NKT // 2, P], f32, tag="xf")
                    nc.sync.dma_start(xf_b[:], xv[:, NKT // 2 :, msl])
                    xb = xb_pool.tile([P, NKT, P], bf16, tag="xb")
                    nc.any.tensor_copy(xb[:, : NKT // 2, :], xf_a[:])
                    nc.any.tensor_copy(xb[:, NKT // 2 :, :], xf_b[:])

                    for n in range(N_NT):
                        ps = psum.tile([P, 512], f32, tag="ps")
                        for i, kt in enumerate(korder):
                            nc.tensor.matmul(
                                ps[:],
                                xb[:, kt, :],
                                wt_t[:, kt, ts(n, 512)],
                                start=(i == 0),
                                stop=(i == NKT - 1),
                            )
                        ysb = ev_pool.tile([P, 512], f32, tag="ysb")
                        nc.vector.tensor_add(
                            ysb[:], ps[:], bias_bc[:, ds(half * HALF + n * 512, 512)]
                        )
                        nc.sync.dma_start(
                            y[msl, ds(half * HALF + n * 512, 512)], ysb[:]
                        )

    nc.compile()
    return nc


def _get_compiled():
    if "nc" not in _COMPILED:
        _COMPILED["nc"] = _build_nc()
    return _COMPILED["nc"]


def _marshal(input, w_packed, w_scale, w_bias, bias):
    import ml_dtypes

    x = np.ascontiguousarray(input, dtype=np.float32).reshape(BS, IN)
    # x^T with rows permuted to plane-major in' order: in' = k*NW + w <- 4w + k
    xt = x.T  # [IN, BS]
    xtp = np.ascontiguousarray(
        xt.reshape(NW, 4, BS).transpose(1, 0, 2).reshape(IN, BS)
    )
    bf16 = ml_dtypes.bfloat16

    def banded(t):
        # [OSH, G] f32 -> [2, 128, 8, HALF] bf16, sbc[h, p, wt, o] = t[h*HALF+o, 2*wt+p//64]
        tT = np.ascontiguousarray(t.T).astype(bf16)  # [G, OSH]
        v = tT.reshape(NW // P, 2, 1, OSH)  # [wt, band, 1, OSH]
        v = np.broadcast_to(v, (NW // P, 2, 64, OSH))  # [wt, band, 64, OSH]
        full = v.transpose(1, 2, 0, 3).reshape(P, NW // P, 2, OSH // 2)
        return np.ascontiguousarray(full.transpose(2, 0, 1, 3))

    in_maps = []
    for c in range(NCORES):
        osl = slice(c * OSH, (c + 1) * OSH)
        wpk16 = w_packed[osl].reshape(OSH, NW).astype(np.int32).view("<u2")[:, ::2]
        in_maps.append(
            {
                "xtp": xtp,
                "wpk": np.ascontiguousarray(wpk16.T),
                "sbc": banded(w_scale[osl].reshape(OSH, G)),
                "bbc": banded(w_bias[osl].reshape(OSH, G)),
                "bias": np.ascontiguousarray(bias[osl].reshape(1, OSH)),
            }
        )
    return in_maps


def kernel(input, w_packed, w_scale, w_bias, bias, _trace=False, _trace_kwargs=None):
    from concourse.bass_utils import run_bass_kernel_spmd

    nc = _get_compiled()
    in_maps = _marshal(input, w_packed, w_scale, w_bias, bias)
    res = run_bass_kernel_spmd(
        nc,
        in_maps,
        core_ids=list(range(NCORES)),
        trace=_trace,
        **(_trace_kwargs or {}),
    )
    ys = [res.results[c]["y"] for c in range(NCORES)]
    out = np.concatenate(ys, axis=1).reshape(B, S, OUT).astype(np.float32)
    if _trace:
        return out, res
    return out


# revision 22
# speedup vs baseline: 1.1226x; 1.1226x over previous
"""GroupQuantLinear on 8 Trainium2 NeuronCores.

y[b,s,o] = x[b,s,:] @ W[o,:] + bias[o], where W is dequantized on-device from
4-bit packed weights with per-(o, group) affine scale/bias (groups of 256 along
the 4096-wide input dim).

Sharding: tensor-parallel on out_features (8 shards of 2048 rows); x replicated.

Per-core kernel (Bass/Tile), v3:
  The packed words only use bits 0..15, so the host passes them as uint16
  [OSH, NW].  Per 1024-col half of OSH:
    - XBAR DMA-transpose (DRAM->SBUF, 8 big [1024,128] u16 chunks) gives
      w16t [word, o] tiles directly in matmul orientation -- no PE transposes
      and no transposition of the 4x larger dequantized data.
    - DVE unpack (shift+and, u16) + 2 broadcast-AP tensor_tensor ops apply the
      per-(o, group) affine dequant: in the [word, o] orientation the group is
      a function of the partition band, so the host pre-replicates scale/bias
      into banded [128, 8wt, OSH] bf16 tensors (layout-only np.broadcast).
      Output goes straight into the SBUF-resident WT [128, 32kt, 1024] bf16.
    - matmul: per m-tile, DMA x^T slice (f32), cast bf16, 64 back-to-back
      [128k,128m]x[128k,512o] matmuls.  The k-accumulation runs in wt-major
      order (kt = plane*8 + wt) so the PE can start as soon as the first
      word-tile is dequantized.  Evict = one DVE add of broadcast bias.

Host marshalling is layout-only: x is transposed/permuted so the contraction
dim lands on SBUF partitions in the same nibble-plane-major order the on-chip
unpack produces (in' = plane*1024 + word); packed words are viewed as uint16;
scale/bias are transposed + replicated into the banded layout.
"""

import numpy as np

B, S, IN, OUT, G = 2, 2048, 4096, 16384, 16
NCORES = 8
OSH = OUT // NCORES       # 2048 out rows per core
BS = B * S                # 4096
NW = IN // 4              # 1024 packed words per out row
P = 128

_COMPILED = {}


def _build_nc():
    from contextlib import ExitStack

    import concourse.bass as bass
    import concourse.mybir as mybir
    import concourse.tile as tile
    from concourse import bacc
    from concourse.bass import ds, ts

    f32 = mybir.dt.float32
    bf16 = mybir.dt.bfloat16
    u16 = mybir.dt.uint16

    nc = bacc.Bacc(None, target_bir_lowering=False)

    xtp = nc.dram_tensor("xtp", [IN, BS], f32, kind="ExternalInput")
    # packed words pre-transposed on host: [word, o]
    wpk = nc.dram_tensor("wpk", [NW, OSH], u16, kind="ExternalInput")
    # [half, 128, 8, HALF] banded scale/bias (bands of 64 partitions per group)
    sbc_d = nc.dram_tensor("sbc", [2, P, 8, OSH // 2], bf16, kind="ExternalInput")
    bbc_d = nc.dram_tensor("bbc", [2, P, 8, OSH // 2], bf16, kind="ExternalInput")
    bias = nc.dram_tensor("bias", [1, OSH], f32, kind="ExternalInput")
    y = nc.dram_tensor("y", [BS, OSH], f32, kind="ExternalOutput")

    NKT = IN // P             # 32 k-subtiles
    NWT = 8                   # word-tiles of 128 words
    HALF = OSH // 2           # 1024 o-cols per half
    N_MT = BS // P            # 32 m-tiles
    N_NT = HALF // 512        # 2 n-tiles of 512 per half

    with tile.TileContext(nc) as tc:
        with ExitStack() as ctx:
            const = ctx.enter_context(tc.tile_pool(name="const", bufs=1))
            wt_pool = ctx.enter_context(tc.tile_pool(name="wt", bufs=1))
            sb_pool = ctx.enter_context(tc.tile_pool(name="sb", bufs=1))
            w16_pool = ctx.enter_context(tc.tile_pool(name="w16", bufs=9))
            q4_pool = ctx.enter_context(tc.tile_pool(name="q4", bufs=3))
            tmp_pool = ctx.enter_context(tc.tile_pool(name="tmp", bufs=3))
            xf_pool = ctx.enter_context(tc.tile_pool(name="xf", bufs=3))
            xb_pool = ctx.enter_context(tc.tile_pool(name="xb", bufs=2))
            ev_pool = ctx.enter_context(tc.tile_pool(name="ev", bufs=3))
            psum = ctx.enter_context(tc.tile_pool(name="psum", bufs=6, space="PSUM"))

            bias_sb = const.tile([1, OSH], f32)
            nc.sync.dma_start(bias_sb[:], bias[:])
            ones_sb = const.tile([1, P], f32)
            nc.any.memset(ones_sb[:], 1.0)
            bias_bc = const.tile([P, OSH], f32)
            for j in range(OSH // 512):
                bps = psum.tile([P, 512], f32, tag="ps")
                nc.tensor.matmul(
                    bps[:], ones_sb[:], bias_sb[:, ts(j, 512)], start=True, stop=True
                )
                nc.any.tensor_copy(bias_bc[:, ts(j, 512)], bps[:])

            # x^T viewed as [p, kt, m] with k-subtile on partitions
            xv = xtp.rearrange("(kt p) m -> p kt m", p=P)

            # wt-major k accumulation order: kt = plane*8 + wt
            korder = [plane * NWT + wt for wt in range(NWT) for plane in range(4)]

            for half in range(2):
                osl = ds(half * HALF, HALF)
                # W^T for this half: [word-in-tile, kt, o] bf16
                wt_t = wt_pool.tile([P, NKT, HALF], bf16, tag="wt")
                sbc = sb_pool.tile([P, NWT, HALF], bf16, tag="sbc")
                bbc = sb_pool.tile([P, NWT, HALF], bf16, tag="bbc")

                # ---- dequant: 8 word-tiles into wt_t, n=0 o-chunk first ----
                w16s = []
                for wt in range(NWT):
                    nc.sync.dma_start(sbc[:, wt, :], sbc_d[half, :, wt, :])
                    nc.sync.dma_start(bbc[:, wt, :], bbc_d[half, :, wt, :])
                    w16 = w16_pool.tile([P, HALF], u16, tag="w16")
                    w16s.append(w16)
                    nc.sync.dma_start(w16[:], wpk[ts(wt, P), osl])
                for oc in range(N_NT):
                    ocs = ts(oc, 512)
                    for wt in range(NWT):
                        q4 = q4_pool.tile([P, 4, 512], u16, tag="q4")
                        for k in range(4):
                            nc.vector.tensor_scalar(
                                q4[:, k, :],
                                w16s[wt][:, ocs],
                                4 * k,
                                0xF,
                                mybir.AluOpType.logical_shift_right,
                                mybir.AluOpType.bitwise_and,
                            )
                        tmp = tmp_pool.tile([P, 4, 512], bf16, tag="tmp")
                        nc.vector.tensor_tensor(
                            tmp[:],
                            q4[:],
                            sbc[:, wt, None, ocs].to_broadcast((P, 4, 512)),
                            mybir.AluOpType.mult,
                        )
                        # kt slices for this wt: plane*8 + wt
                        nc.vector.tensor_tensor(
                            wt_t[:, wt :: NWT, ocs],
                            tmp[:],
                            bbc[:, wt, None, ocs].to_broadcast((P, 4, 512)),
                            mybir.AluOpType.add,
                        )

                # ---- matmul y[:, half] = x @ WT + bias ----
                for mt in range(N_MT):
                    msl = ts(mt, P)
                    xf_a = xf_pool.tile([P, NKT // 2, P], f32, tag="xf")
                    nc.sync.dma_start(xf_a[:], xv[:, : NKT // 2, msl])
                    xf_b = xf_pool.tile([P, NKT // 2, P], f32, tag="xf")
                    nc.sync.dma_start(xf_b[:], xv[:, NKT // 2 :, msl])
                    xb = xb_pool.tile([P, NKT, P], bf16, tag="xb")
                    nc.any.tensor_copy(xb[:, : NKT // 2, :], xf_a[:])
                    nc.any.tensor_copy(xb[:, NKT // 2 :, :], xf_b[:])

                    for n in range(N_NT):
                        ps = psum.tile([P, 512], f32, tag="ps")
                        for i, kt in enumerate(korder):
                            nc.tensor.matmul(
                                ps[:],
                                xb[:, kt, :],
                                wt_t[:, kt, ts(n, 512)],
                                start=(i == 0),
                                stop=(i == NKT - 1),
                            )
                        ysb = ev_pool.tile([P, 512], f32, tag="ysb")
                        nc.vector.tensor_add(
                            ysb[:], ps[:], bias_bc[:, ds(half * HALF + n * 512, 512)]
                        )
                        nc.sync.dma_start(
                            y[msl, ds(half * HALF + n * 512, 512)], ysb[:]
                        )

    nc.compile()
    return nc


def _get_compiled():
    if "nc" not in _COMPILED:
        _COMPILED["nc"] = _build_nc()
    return _COMPILED["nc"]


def _marshal(input, w_packed, w_scale, w_bias, bias):
    import ml_dtypes

    x = np.ascontiguousarray(input, dtype=np.float32).reshape(BS, IN)
    # x^T with rows permuted to plane-major in' order: in' = k*NW + w <- 4w + k
    xt = x.T  # [IN, BS]
    xtp = np.ascontiguousarray(
        xt.reshape(NW, 4, BS).transpose(1, 0, 2).reshape(IN, BS)
    )
    bf16 = ml_dtypes.bfloat16

    def banded(t):
        # [OSH, G] f32 -> [2, 128, 8, HALF] bf16, sbc[h, p, wt, o] = t[h*HALF+o, 2*wt+p//64]
        tT = np.ascontiguousarray(t.T).astype(bf16)  # [G, OSH]
        v = tT.reshape(NW // P, 2, 1, OSH)  # [wt, band, 1, OSH]
        v = np.broadcast_to(v, (NW // P, 2, 64, OSH))  # [wt, band, 64, OSH]
        full = v.transpose(1, 2, 0, 3).reshape(P, NW // P, 2, OSH // 2)
        return np.ascontiguousarray(full.transpose(2, 0, 1, 3))

    in_maps = []
    for c in range(NCORES):
        osl = slice(c * OSH, (c + 1) * OSH)
        wpk16 = w_packed[osl].reshape(OSH, NW).astype(np.int32).view("<u2")[:, ::2]
        in_maps.append(
            {
                "xtp": xtp,
                "wpk": np.ascontiguousarray(wpk16.T),
                "sbc": banded(w_scale[osl].reshape(OSH, G)),
                "bbc": banded(w_bias[osl].reshape(OSH, G)),
                "bias": np.ascontiguousarray(bias[osl].reshape(1, OSH)),
            }
        )
    return in_maps


def kernel(input, w_packed, w_scale, w_bias, bias, _trace=False, _trace_kwargs=None):
    from concourse.bass_utils import run_bass_kernel_spmd

    nc = _get_compiled()
    in_maps = _marshal(input, w_packed, w_scale, w_bias, bias)
    res = run_bass_kernel_spmd(
        nc,
        in_maps,
        core_ids=list(range(NCORES)),
        trace=_trace,
        **(_trace_kwargs or {}),
    )
    ys = [res.results[c]["y"] for c in range(NCORES)]
    out = np.concatenate(ys, axis=1).reshape(B, S, OUT).astype(np.float32)
    if _trace:
        return out, res
    return out
